# revision 22
# baseline (speedup 1.0000x reference)
"""Trainium2 Bass kernel for nn_Attention (B=8, N=1024, C=768, H=12).

Data-parallel over batch: core b handles batch element b.

Math (re-associated to avoid the huge bhqk,bhqd->bkd contraction):
  q = x Wq^T, k = x Wk^T             (per head h: qh, kh  [N, Z])
  S_h = qh kh^T * scale              [N, N]
  E_h = exp(S_h)   (scores are in [-3, 3]; no max-subtraction needed)
  den[qi] = sum_ki E_h[qi, ki]
  ks = kh / den[:, None], qs = qh / den[:, None]
  AT_h = [E_h^T ks ; E_h^T qs]^T     [2Z, N]   (A1T/A2T stacked)
  out  = sum_h AT_h^T @ M_hT + bp    with M_h = [Wq_h;Wk_h] @ Wp^T
         (head-combine and output projection fused on the host)

Structure:
  - natural-layout q/k (for the 1/den scaling) is NOT recomputed by
    matmul; qT/kT round-trip through DRAM and the DMA xbar transposes
    them into natkq[j] while the PE does real work.
  - phase B processes the 12 heads SERIALLY (one at_ps accumulator
    live at a time).  That frees PSUM for a 3-deep score-tile ring,
    which decouples the scores -> exp -> buffer-free latency chain
    that otherwise paces the kernel above the engine-throughput floor.
  - one exp tile per head-phase is computed on the Vector engine via a
    bf16 Schraudolph bit-trick (bitcast_int16(S*K1+K2)); its row-sum
    runs as a DVE reduce.  This sheds ~15% of the Scalar engine load,
    which is the phase-B throughput floor.
  - phase C: F[t] = sum_h AT_h[:,t]^T @ M_hT (fused combine+projection,
    96+96 MMs at the bf16 matmul roofline); bias is added by DVE during
    the PSUM->SBUF copy against a replicated [128, C] bias tile.
  - dummy matmuls warm the PE clock (HAM) during the input-DMA window
    and through the exp-paced final head-phases.

PSUM: psS pool 3 bufs x [128,1024] fp32 (6 banks) for scores /
projection chains / dummies / phase-C F tiles; psA pool 1 buf (2
banks) for the AT accumulator.  SBUF singles freed LIFO between
phases.
"""

import sys
from contextlib import ExitStack

import numpy as np

if "/opt/trn_rl_repo" not in sys.path:
    sys.path.insert(0, "/opt/trn_rl_repo")

import ml_dtypes
import concourse.bass as bass
import concourse.mybir as mybir
import concourse.tile as tile
from concourse import bacc, bass_utils
from concourse.bass import ts

B, N, C, H = 8, 1024, 768, 12
Z = C // H          # 64
P = 128
NT = N // P         # 8 qi tiles
CT = C // P         # 6 c tiles
SCALE = Z ** -0.5   # 0.125
FP = mybir.dt.float32
BF = mybir.dt.bfloat16
FPR = mybir.dt.float32r
I16 = mybir.dt.int16

CCH = [(0, 512), (512, 256)]  # C=768 split into matmul free-dim chunks

# Schraudolph bit-trick exp in bf16: bitcast_int16(round(s*K1 + K2)) is
# bf16(exp(s*SCALE)) with ~+-3% mantissa-interpolation ripple.  The
# ripple is common-mode between E and den (softmax ratio) and averages
# out over the 1024-term q-contraction, so end-to-end error stays
# ~1e-3.  Used to offload part of the exp work from the Scalar engine
# (the phase-B pacer) to the Vector engine.
EXP_K1 = SCALE * np.log2(np.e) * 128.0
EXP_K2 = 16256.0 - 0.0436 * 128.0

last_results = None  # set by kernel() for test harness introspection


def _r(ap):
    """bitcast to float32r for full-rate fp32 matmuls (fp32 data only)."""
    if ap.dtype == FP:
        return ap.bitcast(FPR)
    return ap


def emit(ctx: ExitStack, tc: tile.TileContext, io):
    nc = tc.nc
    xT, wqkT, M, bpr, out = io

    stack = []  # (name, free) in creation order; freed strictly LIFO

    def single(shape, dtype, name):
        t, free = tc.tile(shape, dtype, name=name)
        stack.append((name, free))
        return t

    def free_through(name):
        while stack:
            nm, fr = stack.pop()
            fr()
            if nm == name:
                return
        raise KeyError(name)

    # ---------------- PSUM pools: 3x2 + 1x2 = 8 banks -------------------
    psS = ctx.enter_context(tc.tile_pool(name="psS", bufs=3, space="PSUM"))
    psA = ctx.enter_context(tc.tile_pool(name="psA", bufs=1, space="PSUM"))

    def ps_tile():
        return psS.tile([P, N], FP, name="s", tag="s")

    # SBUF pools (entered before any single so LIFO holds at ctx exit)
    p_E = ctx.enter_context(tc.tile_pool(name="p_E", bufs=14))
    p_kqs = ctx.enter_context(tc.tile_pool(name="p_kqs", bufs=10))
    p_den = ctx.enter_context(tc.tile_pool(name="p_den", bufs=7))
    p_out = ctx.enter_context(tc.tile_pool(name="p_out", bufs=3))

    # ------------- singles, bottom of stack = longest-lived -------------
    M_all = single([P, H * C], BF, name="M_all")
    M_sb = [M_all[:, ts(h, C)] for h in range(H)]
    bp_sb = single([P, C], FP, name="bp_sb")
    AT_sb = [single([P, N], BF, name=f"AT{h}") for h in range(H)]
    # natkq[j]: [128, 2N] cols 0:N = k natural (t-major 128-col blocks),
    # N:2N = q natural; features c of heads 2j, 2j+1.
    natkq = [single([P, 2 * N], BF, name=f"natkq{j}") for j in range(CT)]
    # qT/kT tile j: [128, N] rows = c_out 128j..128j+127 (heads 2j, 2j+1)
    qT_sb = [single([P, N], BF, name=f"qT{j}") for j in range(CT)]
    kT_sb = [single([P, N], BF, name=f"kT{j}") for j in range(CT)]
    wqkT_all = single([P, CT * 2 * C], BF, name="wqkT_all")
    wqkT_sb = [wqkT_all[:, ts(i, 2 * C)] for i in range(CT)]
    xT_all = single([P, CT * N], BF, name="xT_all")
    xT_sb = [xT_all[:, ts(i, N)] for i in range(CT)]

    # DRAM scratch for the qT/kT -> natural-layout xbar transposes
    qkTd = []
    for j in range(CT):
        t_, _free = tc.tile([2, P, N], BF, space="DRAM", name=f"qkTd{j}")
        qkTd.append(t_)

    # HAM keep-warm scratch: the PE clock-gates to 1.2 GHz after ~3.4us
    # of low activity and needs ~3.4us of sustained work to recover;
    # dummy matmuls on a zeroed tile keep it at 2.4 GHz through the
    # input-DMA window and exp-paced stretches with no real PE work.
    warm_sb = single([P, 512], BF, name="warm_sb")
    nc.gpsimd.memset(warm_sb[:], 0)

    def dummy_mms(n):
        ps = ps_tile()
        for i in range(n):
            nc.tensor.matmul(ps[:, 0:512], lhsT=warm_sb[:, 0:P],
                             rhs=warm_sb[:], start=(i == 0), stop=(i == n - 1))

    # ---------------- batched input DMAs (phase-A inputs first) ---------
    for k in range(CT):
        nc.sync.dma_start(xT_sb[k][:], xT[ts(k, P), :])
        nc.sync.dma_start(wqkT_sb[k][:], wqkT[ts(k, P), :])
    # phase-C inputs follow on the same queue (needed only much later);
    # a second hwdge queue tangles the DMA semaphore ring and stalls the
    # input stream, so everything stays on sync.
    nc.sync.dma_start(M_all[:], M[:])
    nc.sync.dma_start(bp_sb[:], bpr[:])

    # ---------------- projection chains ----------------
    def chain(dst_ap, lhsT_of, rhs_of, width):
        """dst_ap = sum_k lhsT_of(k)^T @ rhs_of(k); psum chain + DVE copy."""
        ps = ps_tile()
        for k in range(CT):
            nc.tensor.matmul(
                ps[:, 0:width],
                lhsT=_r(lhsT_of(k)),
                rhs=_r(rhs_of(k)),
                start=(k == 0),
                stop=(k == CT - 1),
            )
        nc.vector.tensor_copy(dst_ap, ps[:, 0:width])

    def qkT_chains(j):
        # k chains + q-ch0 first: pair j's scores t=0..3 become ready one
        # chain earlier (they read kT fully but only qT cols 0:512).
        # One thunk per chain so callers can spread them across t-steps.
        def one(which, ch):
            cols = slice(512 * ch, 512 * ch + 512)
            dst = (qT_sb if which == 0 else kT_sb)[j][:, cols]
            woff = C * which
            chain(dst,
                  lambda k: wqkT_sb[k][:, woff + 128 * j: woff + 128 * j + P],
                  lambda k: xT_sb[k][:, cols], 512)
        return [lambda w=w, c=c: one(w, c) for w, c in
                [(1, 0), (0, 0), (1, 1), (0, 1)]]

    def emit_nat_dma(j):
        """qT/kT[j] -> DRAM -> xbar-transposed natural layout natkq[j]."""
        nc.sync.dma_start(qkTd[j][1], kT_sb[j][:])
        nc.sync.dma_start(qkTd[j][0], qT_sb[j][:])
        nc.sync.dma_start_transpose(
            natkq[j][:, 0:N].rearrange("p (t c) -> p t c", c=P),
            qkTd[j][1].rearrange("c (t q) -> c t q", q=P))
        nc.sync.dma_start_transpose(
            natkq[j][:, N:2 * N].rearrange("p (t c) -> p t c", c=P),
            qkTd[j][0].rearrange("c (t q) -> c t q", q=P))

    # warm the PE during the input-DMA window (no data dependencies), then
    # qT/kT for pair 0 up front so scores/exp start as early as possible
    for _ in range(3):
        dummy_mms(8)
    for th in qkT_chains(0):
        th()
    emit_nat_dma(0)

    # ---------------- phase B: 12 serial head-phases --------------------
    at_queue = []
    LAG = 8

    def drain_at(n):
        while len(at_queue) > n:
            at_queue.pop(0)()

    # extra work emitted inside each head-phase (fills exp-paced slack).
    # NB: trace order defines dependencies -- every producer must be
    # emitted before its first reader.  natkq[j]/qT/kT[j] chains+DMA for
    # pair j+1 are spread over pair j's two head-phases.
    extras = {}
    for j in range(5):
        cthunks = qkT_chains(j + 1)
        extras[2 * j] = [cthunks[0], None, cthunks[1], None,
                         cthunks[2], None, cthunks[3], None]
        extras[2 * j + 1] = [lambda j=j: emit_nat_dma(j + 1)]
    # last pair has no projection work left; dummy matmuls keep HAM warm
    extras[10] = [None, lambda: dummy_mms(4), None, lambda: dummy_mms(4),
                  None, lambda: dummy_mms(4), None, None]
    extras[11] = [None, lambda: dummy_mms(4), None, lambda: dummy_mms(4),
                  None, lambda: dummy_mms(4), None, None]

    # scores are emitted with a 2-step global lookahead (across head-phase
    # seams) so an exp never waits on freshly-issued score matmuls: the
    # 3-deep S ring holds steps i, i+1, i+2.
    steps = [(hp, t) for hp in range(H) for t in range(NT)]
    S_of = {}

    def emit_scores(i):
        hp, t = steps[i]
        j, par = hp // 2, hp & 1
        qt, kt = qT_sb[j], kT_sb[j]
        base = Z * par
        S = ps_tile()
        for ch in range(2):
            cols = slice(512 * ch, 512 * ch + 512)
            nc.tensor.matmul(
                S[:, cols],
                lhsT=qt[base:base + Z, ts(t, P)],
                rhs=kt[base:base + Z, cols],
                start=True, stop=True,
            )
        S_of[i] = S

    emit_scores(0)
    emit_scores(1)
    for hp in range(H):
        j, par = hp // 2, hp & 1
        base = Z * par
        nat3 = natkq[j].rearrange("p (g t c) -> p g t c", g=2, c=P)
        den_t = p_den.tile([P, NT], FP, name="dent")
        rv_t = p_den.tile([P, NT], FP, name="rvt")
        at_ps = psA.tile([P, N], FP, name="at", tag="at")
        ext = list(extras.get(hp, []))
        for t in range(NT):
            i = hp * NT + t
            if i + 2 < len(steps):
                emit_scores(i + 2)
            S = S_of.pop(i)
            E = p_E.tile([P, N], BF, name="Et")
            if hp >= 2 and t in (3, 6):
                # bit-trick exp + row-sum on the Vector engine
                nc.vector.tensor_scalar(
                    E[:].bitcast(I16), S[:], EXP_K1, EXP_K2,
                    op0=mybir.AluOpType.mult, op1=mybir.AluOpType.add)
                nc.vector.tensor_reduce(
                    den_t[:, t:t + 1], E[:],
                    axis=mybir.AxisListType.X, op=mybir.AluOpType.add)
            else:
                nc.scalar.activation(
                    E[:], S[:], mybir.ActivationFunctionType.Exp,
                    scale=SCALE, accum_out=den_t[:, t:t + 1],
                )

            def at_mm(t=t, E=E, at_ps=at_ps, rv_t=rv_t, nat3=nat3, par=par):
                kqs = p_kqs.tile([P, 2 * Z], BF, name="kqst")
                nc.vector.tensor_scalar_mul(
                    kqs[:].rearrange("p (g z) -> p g z", g=2),
                    nat3[:, :, t, ts(par, Z)],
                    rv_t[:, t:t + 1],
                )
                for ch in range(2):
                    cols = slice(512 * ch, 512 * ch + 512)
                    nc.tensor.matmul(
                        at_ps[:, cols],
                        lhsT=kqs[:],
                        rhs=E[:, cols],
                        start=(t == 0), stop=(t == NT - 1),
                    )

            at_queue.append(at_mm)
            drain_at(LAG)
            if t == NT - 1:
                # one batched reciprocal per head-phase; the LAG-deferred
                # at_mm scale ops all run in the next phase, after this
                nc.vector.reciprocal(rv_t[:], den_t[:])
            if ext:
                th = ext.pop(0)
                if th is not None:
                    th()

        def at_copy(hp=hp, at_ps=at_ps):
            # on the Scalar engine: ACT has slack and this keeps the DVE
            # free for the kqs/exp ops that pace the head-phase seams
            nc.scalar.copy(AT_sb[hp][:], at_ps[:])
        at_queue.append(at_copy)
    drain_at(0)

    free_through("natkq0")  # frees xT, wqkT, kT*, qT*, natkq*, warm_sb

    # ---------------- phase C: fused combine + projection + bias ------
    for t in range(NT):
        F_ps = ps_tile()
        for h in range(H):
            for off, w in CCH:
                nc.tensor.matmul(
                    F_ps[:, off:off + w],
                    lhsT=AT_sb[h][:, ts(t, P)],
                    rhs=M_sb[h][:, off:off + w],
                    start=(h == 0), stop=(h == H - 1),
                )
        o = p_out.tile([P, C], FP, name="outt")
        nc.vector.tensor_add(o[:], F_ps[:, 0:C], bp_sb[:])
        nc.sync.dma_start(out[ts(t, P), :], o[:])

    while stack:
        stack.pop()[1]()


def build():
    nc = bacc.Bacc("TRN2", target_bir_lowering=False, debug=False, num_devices=B)
    xT = nc.dram_tensor("xT", [C, N], BF, kind="ExternalInput").ap()
    wqkT = nc.dram_tensor("wqkT", [C, 2 * C], BF, kind="ExternalInput").ap()
    M = nc.dram_tensor("M", [P, H * C], BF, kind="ExternalInput").ap()
    bpr = nc.dram_tensor("bpr", [P, C], FP, kind="ExternalInput").ap()
    out = nc.dram_tensor("out", [N, C], FP, kind="ExternalOutput").ap()
    with tile.TileContext(nc) as tc, ExitStack() as ctx:
        emit(ctx, tc, (xT, wqkT, M, bpr, out))
    nc.compile()
    return nc


def kernel(x, Wq, Wk, Wp, bp, trace=False, **trace_kwargs):
    global last_results
    x = np.asarray(x, dtype=np.float32)
    Wq = np.asarray(Wq, dtype=np.float32)
    Wk = np.asarray(Wk, dtype=np.float32)
    Wp = np.asarray(Wp, dtype=np.float32)
    bp = np.asarray(bp, dtype=np.float32)

    nc = build()
    bf = ml_dtypes.bfloat16
    wqkTc = np.ascontiguousarray(
        np.concatenate([Wq.T, Wk.T], axis=1)).astype(bf)  # [C, 2C]
    # fused combine+projection weights: M_hT = [Wq_h; Wk_h] @ Wp^T  [2Z, C]
    Wq_h = Wq.reshape(H, Z, C)
    Wk_h = Wk.reshape(H, Z, C)
    W2 = np.concatenate([Wq_h, Wk_h], axis=1)             # [H, 2Z, C]
    M_np = np.einsum("hzc,dc->hzd", W2, Wp)               # [H, 2Z, C]
    Mc = np.ascontiguousarray(
        M_np.transpose(1, 0, 2).reshape(P, H * C)).astype(bf)
    bprc = np.ascontiguousarray(
        np.broadcast_to(bp.reshape(1, C), (P, C)).astype(np.float32))
    in_maps = []
    for b in range(B):
        in_maps.append({
            "xT": np.ascontiguousarray(x[b].T).astype(bf),
            "wqkT": wqkTc, "M": Mc, "bpr": bprc,
        })
    res = bass_utils.run_bass_kernel_spmd(
        nc, in_maps, core_ids=list(range(B)), trace=trace, **trace_kwargs)
    last_results = res
    return np.stack([res.results[b]["out"] for b in range(B)], axis=0)


# revision 23
# speedup vs baseline: 1.1614x; 1.1614x over previous
"""Trainium2 Bass kernel for nn_Attention (B=8, N=1024, C=768, H=12).

Data-parallel over batch: core b handles batch element b.

Math (re-associated to avoid the huge bhqk,bhqd->bkd contraction):
  q = x Wq^T, k = x Wk^T             (per head h: qh, kh  [N, Z])
  S_h = qh kh^T * scale              [N, N]
  E_h = exp(S_h)   (scores are in [-3, 3]; no max-subtraction needed)
  den[qi] = sum_ki E_h[qi, ki]
  ks = kh / den[:, None], qs = qh / den[:, None]
  AT_h = [E_h^T ks ; E_h^T qs]^T     [2Z, N]   (A1T/A2T stacked)
  out  = sum_h AT_h^T @ M_hT + bp    with M_h = [Wq_h;Wk_h] @ Wp^T
         (head-combine and output projection fused on the host)

Structure:
  - natural-layout q/k (for the 1/den scaling) is NOT recomputed by
    matmul; qT/kT round-trip through DRAM and the DMA xbar transposes
    them into natkq[j] while the PE does real work.
  - phase B processes the 12 heads SERIALLY (one at_ps accumulator
    live at a time).  That frees PSUM for a 3-deep score-tile ring,
    which decouples the scores -> exp -> buffer-free latency chain
    that otherwise paces the kernel above the engine-throughput floor.
  - one exp tile per head-phase is computed on the Vector engine via a
    bf16 Schraudolph bit-trick (bitcast_int16(S*K1+K2)); its row-sum
    runs as a DVE reduce.  This sheds ~15% of the Scalar engine load,
    which is the phase-B throughput floor.
  - phase C: F[t] = sum_h AT_h[:,t]^T @ M_hT (fused combine+projection,
    96+96 MMs at the bf16 matmul roofline); bias is added by DVE during
    the PSUM->SBUF copy against a replicated [128, C] bias tile.
  - dummy matmuls warm the PE clock (HAM) during the input-DMA window
    and through the exp-paced final head-phases.

PSUM: psS pool 3 bufs x [128,1024] fp32 (6 banks) for scores /
projection chains / dummies / phase-C F tiles; psA pool 1 buf (2
banks) for the AT accumulator.  SBUF singles freed LIFO between
phases.
"""

import sys
from contextlib import ExitStack

import numpy as np

if "/opt/trn_rl_repo" not in sys.path:
    sys.path.insert(0, "/opt/trn_rl_repo")

import ml_dtypes
import concourse.bass as bass
import concourse.mybir as mybir
import concourse.tile as tile
from concourse import bacc, bass_utils
from concourse.bass import ts

B, N, C, H = 8, 1024, 768, 12
Z = C // H          # 64
P = 128
NT = N // P         # 8 qi tiles
CT = C // P         # 6 c tiles
SCALE = Z ** -0.5   # 0.125
FP = mybir.dt.float32
BF = mybir.dt.bfloat16
FPR = mybir.dt.float32r
I16 = mybir.dt.int16

CCH = [(0, 512), (512, 256)]  # C=768 split into matmul free-dim chunks

# Schraudolph bit-trick exp in bf16: bitcast_int16(round(s*K1 + K2)) is
# bf16(exp(s*SCALE)) with ~+-3% mantissa-interpolation ripple.  The
# ripple is common-mode between E and den (softmax ratio) and averages
# out over the 1024-term q-contraction, so end-to-end error stays
# ~1e-3.  Used to offload part of the exp work from the Scalar engine
# (the phase-B pacer) to the Vector engine.
EXP_K1 = SCALE * np.log2(np.e) * 128.0
EXP_K2 = 16256.0 - 0.0436 * 128.0

last_results = None  # set by kernel() for test harness introspection


def _r(ap):
    """bitcast to float32r for full-rate fp32 matmuls (fp32 data only)."""
    if ap.dtype == FP:
        return ap.bitcast(FPR)
    return ap


def emit(ctx: ExitStack, tc: tile.TileContext, io):
    nc = tc.nc
    xT, wqkT, M, bpr, out = io

    stack = []  # (name, free) in creation order; freed strictly LIFO

    def single(shape, dtype, name):
        t, free = tc.tile(shape, dtype, name=name)
        stack.append((name, free))
        return t

    def free_through(name):
        while stack:
            nm, fr = stack.pop()
            fr()
            if nm == name:
                return
        raise KeyError(name)

    # ---------------- PSUM pools: 3x2 + 1x2 = 8 banks -------------------
    psS = ctx.enter_context(tc.tile_pool(name="psS", bufs=3, space="PSUM"))
    psA = ctx.enter_context(tc.tile_pool(name="psA", bufs=1, space="PSUM"))

    def ps_tile():
        return psS.tile([P, N], FP, name="s", tag="s")

    # SBUF pools (entered before any single so LIFO holds at ctx exit)
    p_E = ctx.enter_context(tc.tile_pool(name="p_E", bufs=14))
    p_kqs = ctx.enter_context(tc.tile_pool(name="p_kqs", bufs=10))
    p_den = ctx.enter_context(tc.tile_pool(name="p_den", bufs=7))
    p_out = ctx.enter_context(tc.tile_pool(name="p_out", bufs=3))

    # ------------- singles, bottom of stack = longest-lived -------------
    M_all = single([P, H * C], BF, name="M_all")
    M_sb = [M_all[:, ts(h, C)] for h in range(H)]
    bp_sb = single([P, C], FP, name="bp_sb")
    AT_sb = [single([P, N], BF, name=f"AT{h}") for h in range(H)]
    # natkq[j]: [128, 2N] cols 0:N = k natural (t-major 128-col blocks),
    # N:2N = q natural; features c of heads 2j, 2j+1.
    natkq = [single([P, 2 * N], BF, name=f"natkq{j}") for j in range(CT)]
    # qT/kT tile j: [128, N] rows = c_out 128j..128j+127 (heads 2j, 2j+1)
    qT_sb = [single([P, N], BF, name=f"qT{j}") for j in range(CT)]
    kT_sb = [single([P, N], BF, name=f"kT{j}") for j in range(CT)]
    wqkT_all = single([P, CT * 2 * C], BF, name="wqkT_all")
    wqkT_sb = [wqkT_all[:, ts(i, 2 * C)] for i in range(CT)]
    xT_all = single([P, CT * N], BF, name="xT_all")
    xT_sb = [xT_all[:, ts(i, N)] for i in range(CT)]

    # DRAM scratch for the qT/kT -> natural-layout xbar transposes
    qkTd = []
    for j in range(CT):
        t_, _free = tc.tile([2, P, N], BF, space="DRAM", name=f"qkTd{j}")
        qkTd.append(t_)

    # HAM keep-warm scratch: the PE clock-gates to 1.2 GHz after ~3.4us
    # of low activity and needs ~3.4us of sustained work to recover;
    # dummy matmuls on a zeroed tile keep it at 2.4 GHz through the
    # input-DMA window and exp-paced stretches with no real PE work.
    warm_sb = single([P, 512], BF, name="warm_sb")
    nc.gpsimd.memset(warm_sb[:], 0)

    def dummy_mms(n):
        ps = ps_tile()
        for i in range(n):
            nc.tensor.matmul(ps[:, 0:512], lhsT=warm_sb[:, 0:P],
                             rhs=warm_sb[:], start=(i == 0), stop=(i == n - 1))

    # ---------------- batched input DMAs (phase-A inputs first) ---------
    for k in range(CT):
        nc.sync.dma_start(xT_sb[k][:], xT[ts(k, P), :])
        nc.sync.dma_start(wqkT_sb[k][:], wqkT[ts(k, P), :])
    # phase-C inputs follow on the same queue (needed only much later);
    # a second hwdge queue tangles the DMA semaphore ring and stalls the
    # input stream, so everything stays on sync.
    nc.sync.dma_start(M_all[:], M[:])
    nc.sync.dma_start(bp_sb[:], bpr[:])

    # ---------------- projection chains ----------------
    def chain(dst_ap, lhsT_of, rhs_of, width):
        """dst_ap = sum_k lhsT_of(k)^T @ rhs_of(k); psum chain + DVE copy."""
        ps = ps_tile()
        for k in range(CT):
            nc.tensor.matmul(
                ps[:, 0:width],
                lhsT=_r(lhsT_of(k)),
                rhs=_r(rhs_of(k)),
                start=(k == 0),
                stop=(k == CT - 1),
            )
        nc.vector.tensor_copy(dst_ap, ps[:, 0:width])

    def qkT_chains(j):
        # k chains + q-ch0 first: pair j's scores t=0..3 become ready one
        # chain earlier (they read kT fully but only qT cols 0:512).
        # One thunk per chain so callers can spread them across t-steps.
        def one(which, ch):
            cols = slice(512 * ch, 512 * ch + 512)
            dst = (qT_sb if which == 0 else kT_sb)[j][:, cols]
            woff = C * which
            chain(dst,
                  lambda k: wqkT_sb[k][:, woff + 128 * j: woff + 128 * j + P],
                  lambda k: xT_sb[k][:, cols], 512)
        return [lambda w=w, c=c: one(w, c) for w, c in
                [(1, 0), (0, 0), (1, 1), (0, 1)]]

    def emit_nat_dma(j):
        """qT/kT[j] -> DRAM -> xbar-transposed natural layout natkq[j]."""
        nc.sync.dma_start(qkTd[j][1], kT_sb[j][:])
        nc.sync.dma_start(qkTd[j][0], qT_sb[j][:])
        nc.sync.dma_start_transpose(
            natkq[j][:, 0:N].rearrange("p (t c) -> p t c", c=P),
            qkTd[j][1].rearrange("c (t q) -> c t q", q=P))
        nc.sync.dma_start_transpose(
            natkq[j][:, N:2 * N].rearrange("p (t c) -> p t c", c=P),
            qkTd[j][0].rearrange("c (t q) -> c t q", q=P))

    # warm the PE during the input-DMA window (no data dependencies), then
    # qT/kT for pair 0 up front so scores/exp start as early as possible
    for _ in range(3):
        dummy_mms(8)
    for th in qkT_chains(0):
        th()
    emit_nat_dma(0)

    # ---------------- phase B: 12 serial head-phases --------------------
    at_queue = []
    LAG = 8

    def drain_at(n):
        while len(at_queue) > n:
            at_queue.pop(0)()

    # extra work emitted inside each head-phase (fills exp-paced slack).
    # NB: trace order defines dependencies -- every producer must be
    # emitted before its first reader.  natkq[j]/qT/kT[j] chains+DMA for
    # pair j+1 are spread over pair j's two head-phases.
    extras = {}
    for j in range(5):
        cthunks = qkT_chains(j + 1)
        extras[2 * j] = [cthunks[0], None, cthunks[1], None,
                         cthunks[2], None, cthunks[3], None]
        extras[2 * j + 1] = [lambda j=j: emit_nat_dma(j + 1)]
    # last pair has no projection work left; dummy matmuls keep HAM warm
    extras[10] = [None, lambda: dummy_mms(4), None, lambda: dummy_mms(4),
                  None, lambda: dummy_mms(4), None, None]
    extras[11] = [None, lambda: dummy_mms(4), None, lambda: dummy_mms(4),
                  None, lambda: dummy_mms(4), None, None]

    for hp in range(H):
        j, par = hp // 2, hp & 1
        qt, kt = qT_sb[j], kT_sb[j]
        base = Z * par
        nat3 = natkq[j].rearrange("p (g t c) -> p g t c", g=2, c=P)
        den_t = p_den.tile([P, NT], FP, name="dent")
        rv_t = p_den.tile([P, NT], FP, name="rvt")
        at_ps = psA.tile([P, N], FP, name="at", tag="at")
        ext = list(extras.get(hp, []))
        for t in range(NT):
            S = ps_tile()
            for ch in range(2):
                cols = slice(512 * ch, 512 * ch + 512)
                nc.tensor.matmul(
                    S[:, cols],
                    lhsT=qt[base:base + Z, ts(t, P)],
                    rhs=kt[base:base + Z, cols],
                    start=True, stop=True,
                )
            E = p_E.tile([P, N], BF, name="Et")
            if hp >= 2 and t in (3, 6):
                # bit-trick exp + row-sum on the Vector engine
                nc.vector.tensor_scalar(
                    E[:].bitcast(I16), S[:], EXP_K1, EXP_K2,
                    op0=mybir.AluOpType.mult, op1=mybir.AluOpType.add)
                nc.vector.tensor_reduce(
                    den_t[:, t:t + 1], E[:],
                    axis=mybir.AxisListType.X, op=mybir.AluOpType.add)
            else:
                nc.scalar.activation(
                    E[:], S[:], mybir.ActivationFunctionType.Exp,
                    scale=SCALE, accum_out=den_t[:, t:t + 1],
                )

            def at_mm(t=t, E=E, at_ps=at_ps, rv_t=rv_t, nat3=nat3, par=par):
                kqs = p_kqs.tile([P, 2 * Z], BF, name="kqst")
                nc.vector.tensor_scalar_mul(
                    kqs[:].rearrange("p (g z) -> p g z", g=2),
                    nat3[:, :, t, ts(par, Z)],
                    rv_t[:, t:t + 1],
                )
                for ch in range(2):
                    cols = slice(512 * ch, 512 * ch + 512)
                    nc.tensor.matmul(
                        at_ps[:, cols],
                        lhsT=kqs[:],
                        rhs=E[:, cols],
                        start=(t == 0), stop=(t == NT - 1),
                    )

            at_queue.append(at_mm)
            drain_at(LAG)
            if t == NT - 1:
                # one batched reciprocal per head-phase; the LAG-deferred
                # at_mm scale ops all run in the next phase, after this
                nc.vector.reciprocal(rv_t[:], den_t[:])
            if ext:
                th = ext.pop(0)
                if th is not None:
                    th()

        def at_copy(hp=hp, at_ps=at_ps):
            # on the Scalar engine: ACT has slack and this keeps the DVE
            # free for the kqs/exp ops that pace the head-phase seams
            nc.scalar.copy(AT_sb[hp][:], at_ps[:])
        at_queue.append(at_copy)
    drain_at(0)

    free_through("natkq0")  # frees xT, wqkT, kT*, qT*, natkq*, warm_sb

    # ---------------- phase C: fused combine + projection + bias ------
    for t in range(NT):
        F_ps = ps_tile()
        for h in range(H):
            for off, w in CCH:
                nc.tensor.matmul(
                    F_ps[:, off:off + w],
                    lhsT=AT_sb[h][:, ts(t, P)],
                    rhs=M_sb[h][:, off:off + w],
                    start=(h == 0), stop=(h == H - 1),
                )
        o = p_out.tile([P, C], FP, name="outt")
        nc.vector.tensor_add(o[:], F_ps[:, 0:C], bp_sb[:])
        nc.sync.dma_start(out[ts(t, P), :], o[:])

    while stack:
        stack.pop()[1]()


def build():
    nc = bacc.Bacc("TRN2", target_bir_lowering=False, debug=False, num_devices=B)
    xT = nc.dram_tensor("xT", [C, N], BF, kind="ExternalInput").ap()
    wqkT = nc.dram_tensor("wqkT", [C, 2 * C], BF, kind="ExternalInput").ap()
    M = nc.dram_tensor("M", [P, H * C], BF, kind="ExternalInput").ap()
    bpr = nc.dram_tensor("bpr", [P, C], FP, kind="ExternalInput").ap()
    out = nc.dram_tensor("out", [N, C], FP, kind="ExternalOutput").ap()
    with tile.TileContext(nc) as tc, ExitStack() as ctx:
        emit(ctx, tc, (xT, wqkT, M, bpr, out))
    nc.compile()
    return nc


def kernel(x, Wq, Wk, Wp, bp, trace=False, **trace_kwargs):
    global last_results
    x = np.asarray(x, dtype=np.float32)
    Wq = np.asarray(Wq, dtype=np.float32)
    Wk = np.asarray(Wk, dtype=np.float32)
    Wp = np.asarray(Wp, dtype=np.float32)
    bp = np.asarray(bp, dtype=np.float32)

    nc = build()
    bf = ml_dtypes.bfloat16
    wqkTc = np.ascontiguousarray(
        np.concatenate([Wq.T, Wk.T], axis=1)).astype(bf)  # [C, 2C]
    # fused combine+projection weights: M_hT = [Wq_h; Wk_h] @ Wp^T  [2Z, C]
    Wq_h = Wq.reshape(H, Z, C)
    Wk_h = Wk.reshape(H, Z, C)
    W2 = np.concatenate([Wq_h, Wk_h], axis=1)             # [H, 2Z, C]
    M_np = np.einsum("hzc,dc->hzd", W2, Wp)               # [H, 2Z, C]
    Mc = np.ascontiguousarray(
        M_np.transpose(1, 0, 2).reshape(P, H * C)).astype(bf)
    bprc = np.ascontiguousarray(
        np.broadcast_to(bp.reshape(1, C), (P, C)).astype(np.float32))
    in_maps = []
    for b in range(B):
        in_maps.append({
            "xT": np.ascontiguousarray(x[b].T).astype(bf),
            "wqkT": wqkTc, "M": Mc, "bpr": bprc,
        })
    res = bass_utils.run_bass_kernel_spmd(
        nc, in_maps, core_ids=list(range(B)), trace=trace, **trace_kwargs)
    last_results = res
    return np.stack([res.results[b]["out"] for b in range(B)], axis=0)


# revision 24
# speedup vs baseline: 1.1663x; 1.0042x over previous
"""Trainium2 Bass kernel for nn_Attention (B=8, N=1024, C=768, H=12).

Data-parallel over batch: core b handles batch element b.

Math (re-associated to avoid the huge bhqk,bhqd->bkd contraction):
  q = x Wq^T, k = x Wk^T             (per head h: qh, kh  [N, Z])
  S_h = qh kh^T * scale              [N, N]
  E_h = exp(S_h)   (scores are in [-3, 3]; no max-subtraction needed)
  den[qi] = sum_ki E_h[qi, ki]
  ks = kh / den[:, None], qs = qh / den[:, None]
  AT_h = [E_h^T ks ; E_h^T qs]^T     [2Z, N]   (A1T/A2T stacked)
  out  = sum_h AT_h^T @ M_hT + bp    with M_h = [Wq_h;Wk_h] @ Wp^T
         (head-combine and output projection fused on the host)

Structure:
  - natural-layout q/k (for the 1/den scaling) is NOT recomputed by
    matmul; qT/kT round-trip through DRAM and the DMA xbar transposes
    them into natkq[j] while the PE does real work.
  - phase B processes the 12 heads SERIALLY (one at_ps accumulator
    live at a time).  That frees PSUM for a 3-deep score-tile ring,
    which decouples the scores -> exp -> buffer-free latency chain
    that otherwise paces the kernel above the engine-throughput floor.
  - one exp tile per head-phase is computed on the Vector engine via a
    bf16 Schraudolph bit-trick (bitcast_int16(S*K1+K2)); its row-sum
    runs as a DVE reduce.  This sheds ~15% of the Scalar engine load,
    which is the phase-B throughput floor.
  - phase C: F[t] = sum_h AT_h[:,t]^T @ M_hT (fused combine+projection,
    96+96 MMs at the bf16 matmul roofline); bias is added by DVE during
    the PSUM->SBUF copy against a replicated [128, C] bias tile.
  - dummy matmuls warm the PE clock (HAM) during the input-DMA window
    and through the exp-paced final head-phases.

PSUM: psS pool 3 bufs x [128,1024] fp32 (6 banks) for scores /
projection chains / dummies / phase-C F tiles; psA pool 1 buf (2
banks) for the AT accumulator.  SBUF singles freed LIFO between
phases.
"""

import sys
from contextlib import ExitStack

import numpy as np

if "/opt/trn_rl_repo" not in sys.path:
    sys.path.insert(0, "/opt/trn_rl_repo")

import ml_dtypes
import concourse.bass as bass
import concourse.mybir as mybir
import concourse.tile as tile
from concourse import bacc, bass_utils
from concourse.bass import ts

B, N, C, H = 8, 1024, 768, 12
Z = C // H          # 64
P = 128
NT = N // P         # 8 qi tiles
CT = C // P         # 6 c tiles
SCALE = Z ** -0.5   # 0.125
FP = mybir.dt.float32
BF = mybir.dt.bfloat16
FPR = mybir.dt.float32r
I16 = mybir.dt.int16

CCH = [(0, 512), (512, 256)]  # C=768 split into matmul free-dim chunks

# Schraudolph bit-trick exp in bf16: bitcast_int16(round(s*K1 + K2)) is
# bf16(exp(s*SCALE)) with ~+-3% mantissa-interpolation ripple.  The
# ripple is common-mode between E and den (softmax ratio) and averages
# out over the 1024-term q-contraction, so end-to-end error stays
# ~1e-3.  Used to offload part of the exp work from the Scalar engine
# (the phase-B pacer) to the Vector engine.
EXP_K1 = SCALE * np.log2(np.e) * 128.0
EXP_K2 = 16256.0 - 0.0436 * 128.0

last_results = None  # set by kernel() for test harness introspection


def _r(ap):
    """bitcast to float32r for full-rate fp32 matmuls (fp32 data only)."""
    if ap.dtype == FP:
        return ap.bitcast(FPR)
    return ap


def emit(ctx: ExitStack, tc: tile.TileContext, io):
    nc = tc.nc
    xT, wqkT, M, bpr, out = io

    stack = []  # (name, free) in creation order; freed strictly LIFO

    def single(shape, dtype, name):
        t, free = tc.tile(shape, dtype, name=name)
        stack.append((name, free))
        return t

    def free_through(name):
        while stack:
            nm, fr = stack.pop()
            fr()
            if nm == name:
                return
        raise KeyError(name)

    # ---------------- PSUM pools: 3x2 + 1x2 = 8 banks -------------------
    psS = ctx.enter_context(tc.tile_pool(name="psS", bufs=3, space="PSUM"))
    psA = ctx.enter_context(tc.tile_pool(name="psA", bufs=1, space="PSUM"))

    def ps_tile():
        return psS.tile([P, N], FP, name="s", tag="s")

    # SBUF pools (entered before any single so LIFO holds at ctx exit)
    p_E = ctx.enter_context(tc.tile_pool(name="p_E", bufs=14))
    p_kqs = ctx.enter_context(tc.tile_pool(name="p_kqs", bufs=10))
    p_den = ctx.enter_context(tc.tile_pool(name="p_den", bufs=7))
    p_out = ctx.enter_context(tc.tile_pool(name="p_out", bufs=3))

    # ------------- singles, bottom of stack = longest-lived -------------
    M_all = single([P, H * C], BF, name="M_all")
    M_sb = [M_all[:, ts(h, C)] for h in range(H)]
    bp_sb = single([P, C], FP, name="bp_sb")
    AT_sb = [single([P, N], BF, name=f"AT{h}") for h in range(H)]
    # natkq[j]: [128, 2N] cols 0:N = k natural (t-major 128-col blocks),
    # N:2N = q natural; features c of heads 2j, 2j+1.
    natkq = [single([P, 2 * N], BF, name=f"natkq{j}") for j in range(CT)]
    # qT/kT tile j: [128, N] rows = c_out 128j..128j+127 (heads 2j, 2j+1)
    qT_sb = [single([P, N], BF, name=f"qT{j}") for j in range(CT)]
    kT_sb = [single([P, N], BF, name=f"kT{j}") for j in range(CT)]
    wqkT_all = single([P, CT * 2 * C], BF, name="wqkT_all")
    wqkT_sb = [wqkT_all[:, ts(i, 2 * C)] for i in range(CT)]
    xT_all = single([P, CT * N], BF, name="xT_all")
    xT_sb = [xT_all[:, ts(i, N)] for i in range(CT)]

    # DRAM scratch for the qT/kT -> natural-layout xbar transposes
    qkTd = []
    for j in range(CT):
        t_, _free = tc.tile([2, P, N], BF, space="DRAM", name=f"qkTd{j}")
        qkTd.append(t_)

    # HAM keep-warm scratch: the PE clock-gates to 1.2 GHz after ~3.4us
    # of low activity and needs ~3.4us of sustained work to recover;
    # dummy matmuls on a zeroed tile keep it at 2.4 GHz through the
    # input-DMA window and exp-paced stretches with no real PE work.
    warm_sb = single([P, 512], BF, name="warm_sb")
    nc.gpsimd.memset(warm_sb[:], 0)

    def dummy_mms(n):
        ps = ps_tile()
        for i in range(n):
            nc.tensor.matmul(ps[:, 0:512], lhsT=warm_sb[:, 0:P],
                             rhs=warm_sb[:], start=(i == 0), stop=(i == n - 1))

    # ---------------- batched input DMAs (phase-A inputs first) ---------
    for k in range(CT):
        nc.sync.dma_start(xT_sb[k][:], xT[ts(k, P), :])
        nc.sync.dma_start(wqkT_sb[k][:], wqkT[ts(k, P), :])
    # phase-C inputs follow on the same queue (needed only much later);
    # a second hwdge queue tangles the DMA semaphore ring and stalls the
    # input stream, so everything stays on sync.
    nc.sync.dma_start(M_all[:], M[:])
    nc.sync.dma_start(bp_sb[:], bpr[:])

    # ---------------- projection chains ----------------
    def chain(dst_ap, lhsT_of, rhs_of, width):
        """dst_ap = sum_k lhsT_of(k)^T @ rhs_of(k); psum chain + DVE copy."""
        ps = ps_tile()
        for k in range(CT):
            nc.tensor.matmul(
                ps[:, 0:width],
                lhsT=_r(lhsT_of(k)),
                rhs=_r(rhs_of(k)),
                start=(k == 0),
                stop=(k == CT - 1),
            )
        nc.vector.tensor_copy(dst_ap, ps[:, 0:width])

    def qkT_chains(j):
        # k chains + q-ch0 first: pair j's scores t=0..3 become ready one
        # chain earlier (they read kT fully but only qT cols 0:512).
        # One thunk per chain so callers can spread them across t-steps.
        def one(which, ch):
            cols = slice(512 * ch, 512 * ch + 512)
            dst = (qT_sb if which == 0 else kT_sb)[j][:, cols]
            woff = C * which
            chain(dst,
                  lambda k: wqkT_sb[k][:, woff + 128 * j: woff + 128 * j + P],
                  lambda k: xT_sb[k][:, cols], 512)
        return [lambda w=w, c=c: one(w, c) for w, c in
                [(1, 0), (0, 0), (1, 1), (0, 1)]]

    def emit_nat_dma(j):
        """qT/kT[j] -> DRAM -> xbar-transposed natural layout natkq[j]."""
        nc.sync.dma_start(qkTd[j][1], kT_sb[j][:])
        nc.sync.dma_start(qkTd[j][0], qT_sb[j][:])
        nc.sync.dma_start_transpose(
            natkq[j][:, 0:N].rearrange("p (t c) -> p t c", c=P),
            qkTd[j][1].rearrange("c (t q) -> c t q", q=P))
        nc.sync.dma_start_transpose(
            natkq[j][:, N:2 * N].rearrange("p (t c) -> p t c", c=P),
            qkTd[j][0].rearrange("c (t q) -> c t q", q=P))

    # warm the PE during the input-DMA window (no data dependencies), then
    # qT/kT for pair 0 up front so scores/exp start as early as possible
    for _ in range(3):
        dummy_mms(8)
    for th in qkT_chains(0):
        th()
    emit_nat_dma(0)

    # ---------------- phase B: 12 serial head-phases --------------------
    at_queue = []
    LAG = 8

    def drain_at(n):
        while len(at_queue) > n:
            at_queue.pop(0)()

    # extra work emitted inside each head-phase (fills exp-paced slack).
    # NB: trace order defines dependencies -- every producer must be
    # emitted before its first reader.  natkq[j]/qT/kT[j] chains+DMA for
    # pair j+1 are spread over pair j's two head-phases.
    extras = {}
    for j in range(5):
        cthunks = qkT_chains(j + 1)
        extras[2 * j] = [cthunks[0], None, cthunks[1], None,
                         cthunks[2], None, cthunks[3], None]
        extras[2 * j + 1] = [lambda j=j: emit_nat_dma(j + 1)]
    # last pair has no projection work left; dummy matmuls keep HAM warm
    extras[10] = [None, lambda: dummy_mms(4), None, lambda: dummy_mms(4),
                  None, lambda: dummy_mms(4), None, None]
    extras[11] = [None, lambda: dummy_mms(4), None, lambda: dummy_mms(4),
                  None, lambda: dummy_mms(4), None, None]

    for hp in range(H):
        j, par = hp // 2, hp & 1
        qt, kt = qT_sb[j], kT_sb[j]
        base = Z * par
        nat3 = natkq[j].rearrange("p (g t c) -> p g t c", g=2, c=P)
        den_t = p_den.tile([P, NT], FP, name="dent")
        rv_t = p_den.tile([P, NT], FP, name="rvt")
        at_ps = psA.tile([P, N], FP, name="at", tag="at")
        ext = list(extras.get(hp, []))
        for t in range(NT):
            S = ps_tile()
            for ch in range(2):
                cols = slice(512 * ch, 512 * ch + 512)
                nc.tensor.matmul(
                    S[:, cols],
                    lhsT=qt[base:base + Z, ts(t, P)],
                    rhs=kt[base:base + Z, cols],
                    start=True, stop=True,
                )
            E = p_E.tile([P, N], BF, name="Et")
            if hp >= 2 and t in (2, 5):
                # bit-trick exp + row-sum on the Vector engine
                nc.vector.tensor_scalar(
                    E[:].bitcast(I16), S[:], EXP_K1, EXP_K2,
                    op0=mybir.AluOpType.mult, op1=mybir.AluOpType.add)
                nc.vector.tensor_reduce(
                    den_t[:, t:t + 1], E[:],
                    axis=mybir.AxisListType.X, op=mybir.AluOpType.add)
            else:
                nc.scalar.activation(
                    E[:], S[:], mybir.ActivationFunctionType.Exp,
                    scale=SCALE, accum_out=den_t[:, t:t + 1],
                )

            def at_mm(t=t, E=E, at_ps=at_ps, rv_t=rv_t, nat3=nat3, par=par):
                kqs = p_kqs.tile([P, 2 * Z], BF, name="kqst")
                nc.vector.tensor_scalar_mul(
                    kqs[:].rearrange("p (g z) -> p g z", g=2),
                    nat3[:, :, t, ts(par, Z)],
                    rv_t[:, t:t + 1],
                )
                for ch in range(2):
                    cols = slice(512 * ch, 512 * ch + 512)
                    nc.tensor.matmul(
                        at_ps[:, cols],
                        lhsT=kqs[:],
                        rhs=E[:, cols],
                        start=(t == 0), stop=(t == NT - 1),
                    )

            at_queue.append(at_mm)
            drain_at(LAG)
            if t == NT - 1:
                # one batched reciprocal per head-phase; the LAG-deferred
                # at_mm scale ops all run in the next phase, after this
                nc.vector.reciprocal(rv_t[:], den_t[:])
            if ext:
                th = ext.pop(0)
                if th is not None:
                    th()

        def at_copy(hp=hp, at_ps=at_ps):
            # on the Scalar engine: ACT has slack and this keeps the DVE
            # free for the kqs/exp ops that pace the head-phase seams
            nc.scalar.copy(AT_sb[hp][:], at_ps[:])
        at_queue.append(at_copy)
    drain_at(0)

    free_through("natkq0")  # frees xT, wqkT, kT*, qT*, natkq*, warm_sb

    # ---------------- phase C: fused combine + projection + bias ------
    for t in range(NT):
        F_ps = ps_tile()
        for h in range(H):
            for off, w in CCH:
                nc.tensor.matmul(
                    F_ps[:, off:off + w],
                    lhsT=AT_sb[h][:, ts(t, P)],
                    rhs=M_sb[h][:, off:off + w],
                    start=(h == 0), stop=(h == H - 1),
                )
        o = p_out.tile([P, C], FP, name="outt")
        nc.vector.tensor_add(o[:], F_ps[:, 0:C], bp_sb[:])
        nc.sync.dma_start(out[ts(t, P), :], o[:])

    while stack:
        stack.pop()[1]()


def build():
    nc = bacc.Bacc("TRN2", target_bir_lowering=False, debug=False, num_devices=B)
    xT = nc.dram_tensor("xT", [C, N], BF, kind="ExternalInput").ap()
    wqkT = nc.dram_tensor("wqkT", [C, 2 * C], BF, kind="ExternalInput").ap()
    M = nc.dram_tensor("M", [P, H * C], BF, kind="ExternalInput").ap()
    bpr = nc.dram_tensor("bpr", [P, C], FP, kind="ExternalInput").ap()
    out = nc.dram_tensor("out", [N, C], FP, kind="ExternalOutput").ap()
    with tile.TileContext(nc) as tc, ExitStack() as ctx:
        emit(ctx, tc, (xT, wqkT, M, bpr, out))
    nc.compile()
    return nc


def kernel(x, Wq, Wk, Wp, bp, trace=False, **trace_kwargs):
    global last_results
    x = np.asarray(x, dtype=np.float32)
    Wq = np.asarray(Wq, dtype=np.float32)
    Wk = np.asarray(Wk, dtype=np.float32)
    Wp = np.asarray(Wp, dtype=np.float32)
    bp = np.asarray(bp, dtype=np.float32)

    nc = build()
    bf = ml_dtypes.bfloat16
    wqkTc = np.ascontiguousarray(
        np.concatenate([Wq.T, Wk.T], axis=1)).astype(bf)  # [C, 2C]
    # fused combine+projection weights: M_hT = [Wq_h; Wk_h] @ Wp^T  [2Z, C]
    Wq_h = Wq.reshape(H, Z, C)
    Wk_h = Wk.reshape(H, Z, C)
    W2 = np.concatenate([Wq_h, Wk_h], axis=1)             # [H, 2Z, C]
    M_np = np.einsum("hzc,dc->hzd", W2, Wp)               # [H, 2Z, C]
    Mc = np.ascontiguousarray(
        M_np.transpose(1, 0, 2).reshape(P, H * C)).astype(bf)
    bprc = np.ascontiguousarray(
        np.broadcast_to(bp.reshape(1, C), (P, C)).astype(np.float32))
    in_maps = []
    for b in range(B):
        in_maps.append({
            "xT": np.ascontiguousarray(x[b].T).astype(bf),
            "wqkT": wqkTc, "M": Mc, "bpr": bprc,
        })
    res = bass_utils.run_bass_kernel_spmd(
        nc, in_maps, core_ids=list(range(B)), trace=trace, **trace_kwargs)
    last_results = res
    return np.stack([res.results[b]["out"] for b in range(B)], axis=0)
